# revision 23
# baseline (speedup 1.0000x reference)
"""Trainium2 Bass kernel for nn_BlurredBoundaryAdaptor.

out[b,t,d] = c[b,t,d] + silu(w0[d]*c[b,t-1,d] + w1[d]*c[b,t,d] + w2[d]*c[b,t+1,d] + bias[d])
where c = c_text * mask, mask[b,t] = 0 at dropped phone-boundary positions
(boundaries = cumsum(dur), dropped where drop_rand < 0.8).

Sharding: batch row b -> NeuronCore b (8 rows, 8 cores).

Device layout: the [T, D] row is processed transposed as [D, T] (host
pre-transposes) so that
  - the depthwise conv taps become free-dim slices,
  - per-channel weights become diagonal 128x128 matmuls on the tensor engine
    (fp32r, full rate) accumulating the 3 taps in PSUM,
  - silu + per-channel bias is one scalar-engine activation per tile,
  - the boundary mask is built on-device: cumsum via tensor_tensor_scan,
    drop test on DVE, scatter of zeros via indirect DMA into a DRAM scratch
    row, then gpsimd partition_broadcast to all 128 partitions.
"""

import sys

for _p in ("/opt/trn_rl_repo", "/opt/pypackages"):
    if _p not in sys.path:
        sys.path.insert(0, _p)

import numpy as np

B, T, D, N = 8, 8192, 512, 512
NCORES = 8
DROP_P = 0.8

DG = D // 128          # 4 d-groups of 128 channels
TCH = 2048             # t-chunk (free dim) per tile
NTC = T // TCH         # 4 t-chunks
SUB = 512              # matmul free dim / psum bank
SCRATCH = 16384        # DRAM mask scratch size (>= 2*T)

_CACHE = {}


def _emit_core_pipeline(nc, bass, mybir, pools, io, act):
    """One full pass over the core's row: mask build + masked blur + residual."""
    f32 = mybir.dt.float32
    f32r = mybir.dt.float32r
    i32 = mybir.dt.int32
    Alu = mybir.AluOpType
    mpool, iopool, wpool, ppool = pools
    xT, dur, drop_rand, mask_dram, idx_dram, outT, w_sb, bias_sb = io

    # ---- boundary mask build ----
    dur_i = mpool.tile([1, N], i32, tag="dur_i")
    drop_sb = mpool.tile([1, N], f32, tag="drop")
    nc.sync.dma_start(out=dur_i[:], in_=dur[None, :])
    nc.sync.dma_start(out=drop_sb[:], in_=drop_rand[None, :])
    dur_f = mpool.tile([1, N], f32, tag="dur_f")
    nc.vector.tensor_copy(dur_f[:], dur_i[:])
    bounds = mpool.tile([1, N], f32, tag="bounds")
    # bounds[n] = cumsum(dur)[n]  (exact in fp32: <= 7680)
    nc.vector.tensor_tensor_scan(
        out=bounds[:], data0=dur_f[:], data1=dur_f[:], initial=0.0,
        op0=Alu.add, op1=Alu.bypass)
    # keep = (drop_rand >= DROP_P) -> 1.0 ; idx = bounds + keep*T
    keep = mpool.tile([1, N], f32, tag="keep")
    nc.vector.tensor_scalar(
        out=keep[:], in0=drop_sb[:], scalar1=float(DROP_P), scalar2=None,
        op0=Alu.is_ge)
    idx_f = mpool.tile([1, N], f32, tag="idx_f")
    nc.vector.scalar_tensor_tensor(
        out=idx_f[:], in0=keep[:], scalar=float(T), in1=bounds[:],
        op0=Alu.mult, op1=Alu.add)
    idx_i = mpool.tile([1, N], i32, tag="idx_i")
    nc.vector.tensor_copy(idx_i[:], idx_f[:])
    # HW indirect DMA consumes one offset per PARTITION ([P,1] tables, like
    # tile_scatter_add) — a [1,N] free-dim table is not honored. Round-trip
    # through DRAM to re-layout the 512 indices as [128,4] partition-major.
    nc.sync.dma_start(out=idx_dram[None, :], in_=idx_i[:])
    idx_pm = mpool.tile([128, N // 128], i32, tag="idx_pm")
    nc.sync.dma_start(out=idx_pm[:], in_=idx_dram.rearrange("(j p) -> p j", p=128))

    # ones -> mask_dram (covers whole scratch incl. the keep/overflow region)
    ones_sb = mpool.tile([128, SCRATCH // 128], f32, tag="ones")
    nc.vector.memset(ones_sb[:], 1.0)
    nc.sync.dma_start(
        out=mask_dram.rearrange("(p f) one -> p (f one)", p=128),
        in_=ones_sb[:])
    # scatter zeros at idx (dropped boundaries land in [0,T), kept in [T, 2T))
    zeros_sb = mpool.tile([128, 1], f32, tag="zeros")
    nc.vector.memset(zeros_sb[:], 0.0)
    for j in range(N // 128):
        nc.gpsimd.indirect_dma_start(
            out=mask_dram[:, :],
            out_offset=bass.IndirectOffsetOnAxis(ap=idx_pm[:, j:j + 1], axis=0),
            in_=zeros_sb[:, :],
            in_offset=None)
    # load mask row back and broadcast to 128 partitions with halo:
    # mask_bcast col m corresponds to t = m-1; cols 0 and T+1 are 0.
    mask_row = mpool.tile([1, T], f32, tag="mask_row")
    nc.sync.dma_start(out=mask_row[:], in_=mask_dram[0:T, 0][None, :])
    mask_bcast = mpool.tile([128, T + 2], f32, tag="mask_bcast")
    nc.vector.memset(mask_bcast[:, 0:1], 0.0)
    nc.vector.memset(mask_bcast[:, T + 1:T + 2], 0.0)
    for ci in range(NTC):
        nc.gpsimd.partition_broadcast(
            mask_bcast[:, 1 + ci * TCH:1 + (ci + 1) * TCH],
            mask_row[0:1, ci * TCH:(ci + 1) * TCH])

    # ---- main loop ----
    for ci in range(NTC):
        t0 = ci * TCH
        for g in range(DG):
            x_t = iopool.tile([128, TCH + 2], f32, tag="x")
            # x_t col j <-> t = t0 - 1 + j
            if ci == 0:
                nc.vector.memset(x_t[:, 0:1], 0.0)
                nc.sync.dma_start(
                    out=x_t[:, 1:TCH + 2],
                    in_=xT[g * 128:(g + 1) * 128, 0:TCH + 1])
            elif ci == NTC - 1:
                nc.vector.memset(x_t[:, TCH + 1:TCH + 2], 0.0)
                nc.sync.dma_start(
                    out=x_t[:, 0:TCH + 1],
                    in_=xT[g * 128:(g + 1) * 128, t0 - 1:T])
            else:
                nc.sync.dma_start(
                    out=x_t[:],
                    in_=xT[g * 128:(g + 1) * 128, t0 - 1:t0 + TCH + 1])

            # c = x * mask  (written as fp32r so the PE can consume it at
            # full rate; fp32r is 4-byte and only the conv branch loses
            # the low mantissa bits, the residual keeps c at this rounding)
            c_t = wpool.tile([128, TCH + 2], f32r, tag="c")
            nc.vector.tensor_tensor(
                out=c_t[:], in0=x_t[:], in1=mask_bcast[:, t0:t0 + TCH + 2],
                op=Alu.mult)

            # conv taps as diagonal matmuls (fp32r), accumulated in PSUM
            psum_t = ppool.tile([128, TCH], f32, tag="conv")
            for s in range(TCH // SUB):
                for tap in range(3):
                    nc.tensor.matmul(
                        out=psum_t[:, s * SUB:(s + 1) * SUB],
                        lhsT=w_sb[:, (g * 3 + tap) * 128:(g * 3 + tap + 1) * 128],
                        rhs=c_t[:, s * SUB + tap:s * SUB + tap + SUB],
                        start=(tap == 0), stop=(tap == 2))

            # silu(conv + bias) on the scalar engine
            silu_t = wpool.tile([128, TCH], f32, tag="silu")
            act_func = (mybir.ActivationFunctionType.Silu if act == "silu"
                        else mybir.ActivationFunctionType.Sigmoid)
            nc.scalar.activation(
                out=silu_t[:], in_=psum_t[:],
                func=act_func,
                bias=bias_sb[:, g:g + 1], scale=1.0)

            # residual add + store
            out_t = iopool.tile([128, TCH], f32, tag="out")
            nc.vector.tensor_tensor(
                out=out_t[:], in0=silu_t[:], in1=c_t[:, 1:TCH + 1],
                op=Alu.add)
            nc.sync.dma_start(
                out=outT[g * 128:(g + 1) * 128, t0:t0 + TCH], in_=out_t[:])


def _build_program(act="silu", passes=1):
    import concourse.bacc as bacc
    import concourse.tile as tile
    import concourse.mybir as mybir
    from concourse import bass

    f32 = mybir.dt.float32
    i32 = mybir.dt.int32

    nc = bacc.Bacc("TRN2", target_bir_lowering=False, debug=False,
                   enable_asserts=True, num_devices=1)

    xT = nc.dram_tensor("xT", [D, T], f32, kind="ExternalInput").ap()
    dur = nc.dram_tensor("dur", [N], i32, kind="ExternalInput").ap()
    drop_rand = nc.dram_tensor("drop_rand", [N], f32, kind="ExternalInput").ap()
    wdiag = nc.dram_tensor("wdiag", [DG * 3, 128, 128], mybir.dt.float32r,
                           kind="ExternalInput").ap()
    bias_pg = nc.dram_tensor("bias_pg", [128, DG], f32, kind="ExternalInput").ap()
    outT = nc.dram_tensor("outT", [D, T], f32, kind="ExternalOutput").ap()
    # Internal scratch row for the scattered mask (flat [SCRATCH] as [SCRATCH,1]).
    mask_dram = nc.dram_tensor("mask_scratch", [SCRATCH, 1], f32, kind="Internal").ap()
    idx_dram = nc.dram_tensor("idx_scratch", [N], i32, kind="Internal").ap()

    with tile.TileContext(nc) as tc:
        with (
            tc.tile_pool(name="const", bufs=1) as cpool,
            tc.tile_pool(name="mask", bufs=1) as mpool,
            tc.tile_pool(name="io", bufs=3) as iopool,
            tc.tile_pool(name="work", bufs=3) as wpool,
            tc.tile_pool(name="psum", bufs=2, space="PSUM") as ppool,
        ):
            # ---- constants ----
            w_sb = cpool.tile([128, DG * 3 * 128], mybir.dt.float32r, tag="weights")
            for i in range(DG * 3):
                nc.sync.dma_start(out=w_sb[:, i * 128:(i + 1) * 128], in_=wdiag[i])
            bias_sb = cpool.tile([128, DG], f32, tag="bias")
            nc.sync.dma_start(out=bias_sb[:], in_=bias_pg[:, :])

            pools = (mpool, iopool, wpool, ppool)
            io = (xT, dur, drop_rand, mask_dram, idx_dram, outT, w_sb, bias_sb)
            for _ in range(passes):
                _emit_core_pipeline(nc, bass, mybir, pools, io, act)

    nc.compile()
    return nc


def _get_nc(act="silu", passes=1):
    key = (act, passes)
    if key not in _CACHE:
        _CACHE[key] = _build_program(act, passes)
    return _CACHE[key]


def _host_prep(c_text, dur, drop_rand, blur_w, blur_b):
    """Per-core input maps. Weights -> per-group diagonal lhsT matrices."""
    xT = np.ascontiguousarray(c_text.transpose(0, 2, 1)).astype(np.float32)  # [B,D,T]
    w = np.asarray(blur_w, dtype=np.float32).reshape(D, 3)
    wd = np.zeros((DG * 3, 128, 128), dtype=np.float32)
    for g in range(DG):
        for tap in range(3):
            np.fill_diagonal(wd[g * 3 + tap], w[g * 128:(g + 1) * 128, tap])
    bias_pg = np.ascontiguousarray(
        np.asarray(blur_b, dtype=np.float32).reshape(DG, 128).T)  # [128, DG]
    in_maps = []
    for b in range(B):
        in_maps.append({
            "xT": xT[b],
            "dur": np.ascontiguousarray(dur[b]).astype(np.int32),
            "drop_rand": np.ascontiguousarray(drop_rand[b]).astype(np.float32),
            "wdiag": wd,
            "bias_pg": bias_pg,
        })
    return in_maps


def kernel(c_text, dur, drop_rand, blur_w, blur_b):
    from concourse.bass_utils import run_bass_kernel_spmd

    nc = _get_nc()
    in_maps = _host_prep(c_text, dur, drop_rand, blur_w, blur_b)
    r = run_bass_kernel_spmd(nc, in_maps, core_ids=list(range(NCORES)))
    out = np.stack([r.results[b]["outT"] for b in range(B)])  # [B, D, T]
    return np.ascontiguousarray(out.transpose(0, 2, 1))


# ---------------------------------------------------------------------------
# Timing support (used by test.py, not by the grading harness).
# ---------------------------------------------------------------------------

def _make_timed_fn(nc, in_maps, reduce_outputs=True):
    """Sharded jitted callable over 8 cores with device-resident buffers so it
    can be re-dispatched for timing."""
    import jax
    import numpy as _np
    import concourse.mybir as mybir
    from jax.sharding import Mesh, PartitionSpec, NamedSharding
    from jax.experimental.shard_map import shard_map
    from concourse.bass2jax import (_bass_exec_p, install_neuronx_cc_hook,
                                    partition_id_tensor)

    install_neuronx_cc_hook()
    n_cores = len(in_maps)
    partition_name = nc.partition_id_tensor.name if nc.partition_id_tensor else None
    in_names, out_names, out_avals, zero_outs = [], [], [], []
    for alloc in nc.m.functions[0].allocations:
        if not isinstance(alloc, mybir.MemoryLocationSet):
            continue
        name = alloc.memorylocations[0].name
        if alloc.kind == "ExternalInput":
            if name != partition_name:
                in_names.append(name)
        elif alloc.kind == "ExternalOutput":
            shape = tuple(alloc.tensor_shape)
            dtype = mybir.dt.np(alloc.dtype)
            out_names.append(name)
            out_avals.append(jax.core.ShapedArray(shape, dtype))
            zero_outs.append(_np.zeros(shape, dtype))
    n_params = len(in_names)
    all_in_names = list(in_names) + list(out_names)
    if partition_name is not None:
        all_in_names.append(partition_name)

    def _body(*args):
        ops = list(args)
        if partition_name is not None:
            ops.append(partition_id_tensor())
        outs = _bass_exec_p.bind(
            *ops,
            out_avals=tuple(out_avals),
            in_names=tuple(all_in_names),
            out_names=tuple(out_names),
            lowering_input_output_aliases=(),
            sim_require_finite=True,
            sim_require_nnan=True,
            nc=nc,
        )
        return tuple(outs)

    devices = jax.devices()[:n_cores]
    mesh = Mesh(np.asarray(devices), ("core",))
    in_specs = (PartitionSpec("core"),) * (n_params + len(out_names))
    out_specs = (PartitionSpec("core"),) * len(out_names)
    fn = jax.jit(shard_map(_body, mesh=mesh, in_specs=in_specs,
                           out_specs=out_specs, check_rep=False),
                 keep_unused=True)
    concat_in = [
        np.concatenate([np.asarray(in_maps[c][nm])[None] for c in range(n_cores)],
                       axis=0).reshape(n_cores * np.asarray(in_maps[0][nm]).shape[0],
                                       *np.asarray(in_maps[0][nm]).shape[1:])
        for nm in in_names
    ]
    sharding = NamedSharding(mesh, PartitionSpec("core"))
    dev_in = [jax.device_put(a, sharding) for a in concat_in]

    def make_zero_sets(n):
        # output buffers get consumed (aliased into NEFF outputs) per call —
        # pre-stage one set per timing iteration
        return [
            [jax.device_put(
                np.zeros((n_cores * z.shape[0], *z.shape[1:]), z.dtype), sharding)
             for z in zero_outs]
            for _ in range(n)
        ]

    if reduce_outputs:
        # only a [8]-vector of per-shard sums crosses the axon relay: blocking
        # on the raw 16MB/core outputs marshals them to the client (~74ms/call)
        import jax.numpy as jnp
        inner = fn

        def _sums(*args):
            outs = inner(*args)
            return jax.jit(shard_map(
                lambda *os: tuple(jnp.sum(o, keepdims=True)[:, 0] for o in os),
                mesh=mesh,
                in_specs=(PartitionSpec("core"),) * len(outs),
                out_specs=(PartitionSpec("core"),) * len(outs),
                check_rep=False))(*outs)

        fn = _sums

    return fn, dev_in, make_zero_sets


def _time_fn(fn, dev_in, make_zero_sets, iters):
    # Serialized dispatch (block per call): axon's mesh desyncs under deep
    # async pipelines. Per-call overhead is cancelled by the marginal-pass
    # differential in time_kernel.
    import time as _t
    import jax
    zsets = make_zero_sets(iters + 1)
    jax.block_until_ready(fn(*dev_in, *zsets[0]))
    times = []
    for i in range(iters):
        t0 = _t.perf_counter()
        jax.block_until_ready(fn(*dev_in, *zsets[i + 1]))
        times.append(_t.perf_counter() - t0)
    times.sort()
    k = max(1, len(times) // 3)
    return sum(times[:k]) / k


def time_kernel(inputs, iters=20, passes=16):
    """Marginal per-pass device time: (t(passes) - t(1)) / (passes - 1).

    Cancels per-dispatch overhead (axon round trip, NEFF launch, transfers).
    """
    in_maps = _host_prep(**inputs)
    nc1 = _get_nc("silu", 1)
    fn1, in1, mz1 = _make_timed_fn(nc1, in_maps)
    t1 = _time_fn(fn1, in1, mz1, iters)
    ncK = _get_nc("silu", passes)
    fnK, inK, mzK = _make_timed_fn(ncK, in_maps)
    tK = _time_fn(fnK, inK, mzK, iters)
    per_pass = (tK - t1) / (passes - 1)
    print(f"t(1 pass)={t1*1e6:.1f}us  t({passes} passes)={tK*1e6:.1f}us  "
          f"marginal per-pass={per_pass*1e6:.1f}us")
    return per_pass * 1e9


# revision 35
# speedup vs baseline: 2.2078x; 2.2078x over previous
"""Trainium2 Bass kernel for nn_BlurredBoundaryAdaptor.

out[b,t,d] = c[b,t,d] + silu(w0[d]*c[b,t-1,d] + w1[d]*c[b,t,d] + w2[d]*c[b,t+1,d] + bias[d])
where c = c_text * mask, mask[b,t] = 0 at dropped phone-boundary positions
(boundaries = cumsum(dur), dropped where drop_rand < 0.8).

Sharding: batch row b -> NeuronCore b (8 rows, 8 cores).

Device layout: the [T, D] row is processed transposed as [D, T] (host
pre-transposes) so that
  - the depthwise conv taps become free-dim slices,
  - per-channel weights become diagonal 128x128 matmuls on the tensor engine
    (fp32r, full rate) accumulating the 3 taps in PSUM,
  - silu + per-channel bias is one scalar-engine activation per tile,
  - the boundary mask is built on-device: cumsum via tensor_tensor_scan,
    drop test on DVE, scatter of zeros via indirect DMA into a DRAM scratch
    row, then gpsimd partition_broadcast to all 128 partitions.
"""

import sys

for _p in ("/opt/trn_rl_repo", "/opt/pypackages"):
    if _p not in sys.path:
        sys.path.insert(0, _p)

import numpy as np

B, T, D, N = 8, 8192, 512, 512
NCORES = 8
DROP_P = 0.8

DG = D // 128          # 4 d-groups of 128 channels
TCH = 4096             # t-chunk (free dim) per tile
NTC = T // TCH         # t-chunks
SUB = 512              # matmul free dim / psum bank
PS = 2048              # psum tile free dim (4 banks)
SCRATCH = 16384        # DRAM mask scratch size (>= 2*T)

_CACHE = {}


def _emit_core_pipeline(nc, bass, mybir, pools, io, act):
    """One full pass over the core's row: mask build + masked blur + residual."""
    f32 = mybir.dt.float32
    f32r = mybir.dt.float32r
    i32 = mybir.dt.int32
    Alu = mybir.AluOpType
    mpool, iopool, wpool, ppool = pools
    xT, dur, drop_rand, mask_dram, outT, w_sb, bias_sb, ltri_sb = io
    KB = N // 128  # boundaries per partition (4)

    # ---- boundary mask build (partition-major: boundary n=4p+k at [p,k]) ----
    # HW indirect DMA consumes one offset per PARTITION, so indices are
    # computed directly in [128, KB] layout: per-partition prefix scan of the
    # 4-element blocks + strict-lower-triangular matmul for the block offsets.
    dur_i = mpool.tile([128, KB], i32, tag="dur_i")
    drop_sb = mpool.tile([128, KB], f32, tag="drop")
    nc.sync.dma_start(out=dur_i[:], in_=dur.rearrange("(p k) -> p k", k=KB))
    nc.sync.dma_start(out=drop_sb[:], in_=drop_rand.rearrange("(p k) -> p k", k=KB))
    dur_f = mpool.tile([128, KB], f32, tag="dur_f")
    nc.vector.tensor_copy(dur_f[:], dur_i[:])
    scan = mpool.tile([128, KB], f32, tag="scan")
    nc.vector.tensor_tensor_scan(
        out=scan[:], data0=dur_f[:], data1=dur_f[:], initial=0.0,
        op0=Alu.add, op1=Alu.bypass)
    # offs[p] = sum of block totals of partitions q < p
    offs_ps = ppool.tile([128, 1], f32, tag="conv")
    nc.tensor.matmul(out=offs_ps[:], lhsT=ltri_sb[:], rhs=scan[:, KB - 1:KB],
                     start=True, stop=True)
    offs = mpool.tile([128, 1], f32, tag="offs_sb")
    nc.vector.tensor_copy(offs[:], offs_ps[:])
    bounds = mpool.tile([128, KB], f32, tag="bounds")
    nc.vector.tensor_scalar(
        out=bounds[:], in0=scan[:], scalar1=offs[:, 0:1], scalar2=None,
        op0=Alu.add)
    # keep = (drop_rand >= DROP_P) -> 1.0 ; idx = bounds + keep*T
    keep = mpool.tile([128, KB], f32, tag="keep")
    nc.vector.tensor_scalar(
        out=keep[:], in0=drop_sb[:], scalar1=float(DROP_P), scalar2=None,
        op0=Alu.is_ge)
    idx_f = mpool.tile([128, KB], f32, tag="idx_f")
    nc.vector.scalar_tensor_tensor(
        out=idx_f[:], in0=keep[:], scalar=float(T), in1=bounds[:],
        op0=Alu.mult, op1=Alu.add)
    idx_pm = mpool.tile([128, KB], i32, tag="idx_pm")
    nc.vector.tensor_copy(idx_pm[:], idx_f[:])

    # ones -> mask_dram (covers whole scratch incl. the keep/overflow region)
    ones_sb = mpool.tile([128, SCRATCH // 128], f32, tag="ones")
    nc.vector.memset(ones_sb[:], 1.0)
    nc.sync.dma_start(
        out=mask_dram.rearrange("(p f) one -> p (f one)", p=128),
        in_=ones_sb[:])
    # scatter zeros at idx (dropped boundaries land in [0,T), kept in [T, 2T))
    zeros_sb = mpool.tile([128, 1], f32, tag="zeros")
    nc.vector.memset(zeros_sb[:], 0.0)
    for j in range(KB):
        nc.gpsimd.indirect_dma_start(
            out=mask_dram[:, :],
            out_offset=bass.IndirectOffsetOnAxis(ap=idx_pm[:, j:j + 1], axis=0),
            in_=zeros_sb[:, :],
            in_offset=None)
    # load mask row back and broadcast to 128 partitions with halo:
    # mask_bcast col m corresponds to t = m-1; cols 0 and T+1 are 0.
    mask_row = mpool.tile([1, T], f32, tag="mask_row")
    nc.sync.dma_start(out=mask_row[:], in_=mask_dram[0:T, 0][None, :])
    mask_bcast = mpool.tile([128, T + 2], f32, tag="mask_bcast")
    nc.vector.memset(mask_bcast[:, 0:1], 0.0)
    nc.vector.memset(mask_bcast[:, T + 1:T + 2], 0.0)
    for ci in range(NTC):
        nc.gpsimd.partition_broadcast(
            mask_bcast[:, 1 + ci * TCH:1 + (ci + 1) * TCH],
            mask_row[0:1, ci * TCH:(ci + 1) * TCH])

    # ---- main loop ----
    for ci in range(NTC):
        t0 = ci * TCH
        for g in range(DG):
            x_t = iopool.tile([128, TCH + 2], f32, tag="x")
            # x_t col j <-> t = t0 - 1 + j
            if ci == 0:
                nc.vector.memset(x_t[:, 0:1], 0.0)
                nc.sync.dma_start(
                    out=x_t[:, 1:TCH + 2],
                    in_=xT[g * 128:(g + 1) * 128, 0:TCH + 1])
            elif ci == NTC - 1:
                nc.vector.memset(x_t[:, TCH + 1:TCH + 2], 0.0)
                nc.sync.dma_start(
                    out=x_t[:, 0:TCH + 1],
                    in_=xT[g * 128:(g + 1) * 128, t0 - 1:T])
            else:
                nc.sync.dma_start(
                    out=x_t[:],
                    in_=xT[g * 128:(g + 1) * 128, t0 - 1:t0 + TCH + 1])

            # c = x * mask  (written as fp32r so the PE can consume it at
            # full rate; fp32r is 4-byte and only the conv branch loses
            # the low mantissa bits, the residual keeps c at this rounding)
            c_t = wpool.tile([128, TCH + 2], f32r, tag="c")
            nc.vector.tensor_tensor(
                out=c_t[:], in0=x_t[:], in1=mask_bcast[:, t0:t0 + TCH + 2],
                op=Alu.mult)

            # conv taps as diagonal matmuls (fp32r), accumulated in PSUM
            # (PS-column halves: a psum tile is 4 banks, 2 in flight)
            act_func = (mybir.ActivationFunctionType.Silu if act == "silu"
                        else mybir.ActivationFunctionType.Sigmoid)
            out_t = iopool.tile([128, TCH], f32, tag="out")
            for h in range(TCH // PS):
                psum_t = ppool.tile([128, PS], f32, tag="conv")
                for s in range(PS // SUB):
                    base = h * PS + s * SUB
                    for tap in range(3):
                        nc.tensor.matmul(
                            out=psum_t[:, s * SUB:(s + 1) * SUB],
                            lhsT=w_sb[:, (g * 3 + tap) * 128:(g * 3 + tap + 1) * 128],
                            rhs=c_t[:, base + tap:base + tap + SUB],
                            start=(tap == 0), stop=(tap == 2))
                # silu(conv + bias) on the scalar engine, straight into out_t
                nc.scalar.activation(
                    out=out_t[:, h * PS:(h + 1) * PS], in_=psum_t[:],
                    func=act_func,
                    bias=bias_sb[:, g:g + 1], scale=1.0)

            # in-place residual add + store (split across DVE and GPSIMD to
            # keep the vector engine under the DMA roofline)
            res_eng = nc.vector if (g % 2 == 0) else nc.gpsimd
            res_eng.tensor_tensor(
                out=out_t[:], in0=out_t[:], in1=c_t[:, 1:TCH + 1],
                op=Alu.add)
            nc.sync.dma_start(
                out=outT[g * 128:(g + 1) * 128, t0:t0 + TCH], in_=out_t[:])


def _build_program(act="silu", passes=1):
    import concourse.bacc as bacc
    import concourse.tile as tile
    import concourse.mybir as mybir
    from concourse import bass

    f32 = mybir.dt.float32
    i32 = mybir.dt.int32

    nc = bacc.Bacc("TRN2", target_bir_lowering=False, debug=False,
                   enable_asserts=True, num_devices=1)

    xT = nc.dram_tensor("xT", [D, T], f32, kind="ExternalInput").ap()
    dur = nc.dram_tensor("dur", [N], i32, kind="ExternalInput").ap()
    drop_rand = nc.dram_tensor("drop_rand", [N], f32, kind="ExternalInput").ap()
    wdiag = nc.dram_tensor("wdiag", [DG * 3, 128, 128], mybir.dt.float32r,
                           kind="ExternalInput").ap()
    bias_pg = nc.dram_tensor("bias_pg", [128, DG], f32, kind="ExternalInput").ap()
    outT = nc.dram_tensor("outT", [D, T], f32, kind="ExternalOutput").ap()
    ltri = nc.dram_tensor("ltri", [128, 128], f32, kind="ExternalInput").ap()
    # Internal scratch row for the scattered mask (flat [SCRATCH] as [SCRATCH,1]).
    mask_dram = nc.dram_tensor("mask_scratch", [SCRATCH, 1], f32, kind="Internal").ap()

    with tile.TileContext(nc) as tc:
        with (
            tc.tile_pool(name="const", bufs=1) as cpool,
            tc.tile_pool(name="mask", bufs=1) as mpool,
            tc.tile_pool(name="io", bufs=2) as iopool,
            tc.tile_pool(name="work", bufs=2) as wpool,
            tc.tile_pool(name="psum", bufs=2, space="PSUM") as ppool,
        ):
            # ---- constants ----
            w_sb = cpool.tile([128, DG * 3 * 128], mybir.dt.float32r, tag="weights")
            for i in range(DG * 3):
                nc.sync.dma_start(out=w_sb[:, i * 128:(i + 1) * 128], in_=wdiag[i])
            bias_sb = cpool.tile([128, DG], f32, tag="bias")
            nc.sync.dma_start(out=bias_sb[:], in_=bias_pg[:, :])
            ltri_sb = cpool.tile([128, 128], f32, tag="ltri")
            nc.sync.dma_start(out=ltri_sb[:], in_=ltri[:, :])

            pools = (mpool, iopool, wpool, ppool)
            io = (xT, dur, drop_rand, mask_dram, outT, w_sb, bias_sb, ltri_sb)
            for _ in range(passes):
                _emit_core_pipeline(nc, bass, mybir, pools, io, act)

    nc.compile()
    return nc


def _get_nc(act="silu", passes=1):
    key = (act, passes)
    if key not in _CACHE:
        _CACHE[key] = _build_program(act, passes)
    return _CACHE[key]


def _host_prep(c_text, dur, drop_rand, blur_w, blur_b):
    """Per-core input maps. Weights -> per-group diagonal lhsT matrices."""
    xT = np.ascontiguousarray(c_text.transpose(0, 2, 1)).astype(np.float32)  # [B,D,T]
    w = np.asarray(blur_w, dtype=np.float32).reshape(D, 3)
    wd = np.zeros((DG * 3, 128, 128), dtype=np.float32)
    for g in range(DG):
        for tap in range(3):
            np.fill_diagonal(wd[g * 3 + tap], w[g * 128:(g + 1) * 128, tap])
    bias_pg = np.ascontiguousarray(
        np.asarray(blur_b, dtype=np.float32).reshape(DG, 128).T)  # [128, DG]
    # ltri[q, p] = 1 iff q < p  (lhsT for the block-offset matmul)
    ltri = np.triu(np.ones((128, 128), np.float32), k=1)
    in_maps = []
    for b in range(B):
        in_maps.append({
            "xT": xT[b],
            "dur": np.ascontiguousarray(dur[b]).astype(np.int32),
            "drop_rand": np.ascontiguousarray(drop_rand[b]).astype(np.float32),
            "wdiag": wd,
            "bias_pg": bias_pg,
            "ltri": ltri,
        })
    return in_maps


def kernel(c_text, dur, drop_rand, blur_w, blur_b):
    from concourse.bass_utils import run_bass_kernel_spmd

    nc = _get_nc()
    in_maps = _host_prep(c_text, dur, drop_rand, blur_w, blur_b)
    r = run_bass_kernel_spmd(nc, in_maps, core_ids=list(range(NCORES)))
    out = np.stack([r.results[b]["outT"] for b in range(B)])  # [B, D, T]
    return np.ascontiguousarray(out.transpose(0, 2, 1))


# ---------------------------------------------------------------------------
# Timing support (used by test.py, not by the grading harness).
# ---------------------------------------------------------------------------

def _make_timed_fn(nc, in_maps, reduce_outputs=True):
    """Sharded jitted callable over 8 cores with device-resident buffers so it
    can be re-dispatched for timing."""
    import jax
    import numpy as _np
    import concourse.mybir as mybir
    from jax.sharding import Mesh, PartitionSpec, NamedSharding
    from jax.experimental.shard_map import shard_map
    from concourse.bass2jax import (_bass_exec_p, install_neuronx_cc_hook,
                                    partition_id_tensor)

    install_neuronx_cc_hook()
    n_cores = len(in_maps)
    partition_name = nc.partition_id_tensor.name if nc.partition_id_tensor else None
    in_names, out_names, out_avals, zero_outs = [], [], [], []
    for alloc in nc.m.functions[0].allocations:
        if not isinstance(alloc, mybir.MemoryLocationSet):
            continue
        name = alloc.memorylocations[0].name
        if alloc.kind == "ExternalInput":
            if name != partition_name:
                in_names.append(name)
        elif alloc.kind == "ExternalOutput":
            shape = tuple(alloc.tensor_shape)
            dtype = mybir.dt.np(alloc.dtype)
            out_names.append(name)
            out_avals.append(jax.core.ShapedArray(shape, dtype))
            zero_outs.append(_np.zeros(shape, dtype))
    n_params = len(in_names)
    all_in_names = list(in_names) + list(out_names)
    if partition_name is not None:
        all_in_names.append(partition_name)

    def _body(*args):
        ops = list(args)
        if partition_name is not None:
            ops.append(partition_id_tensor())
        outs = _bass_exec_p.bind(
            *ops,
            out_avals=tuple(out_avals),
            in_names=tuple(all_in_names),
            out_names=tuple(out_names),
            lowering_input_output_aliases=(),
            sim_require_finite=True,
            sim_require_nnan=True,
            nc=nc,
        )
        return tuple(outs)

    devices = jax.devices()[:n_cores]
    mesh = Mesh(np.asarray(devices), ("core",))
    in_specs = (PartitionSpec("core"),) * (n_params + len(out_names))
    out_specs = (PartitionSpec("core"),) * len(out_names)
    fn = jax.jit(shard_map(_body, mesh=mesh, in_specs=in_specs,
                           out_specs=out_specs, check_rep=False),
                 keep_unused=True)
    concat_in = [
        np.concatenate([np.asarray(in_maps[c][nm])[None] for c in range(n_cores)],
                       axis=0).reshape(n_cores * np.asarray(in_maps[0][nm]).shape[0],
                                       *np.asarray(in_maps[0][nm]).shape[1:])
        for nm in in_names
    ]
    sharding = NamedSharding(mesh, PartitionSpec("core"))
    dev_in = [jax.device_put(a, sharding) for a in concat_in]

    def make_zero_sets(n):
        # output buffers get consumed (aliased into NEFF outputs) per call —
        # pre-stage one set per timing iteration
        return [
            [jax.device_put(
                np.zeros((n_cores * z.shape[0], *z.shape[1:]), z.dtype), sharding)
             for z in zero_outs]
            for _ in range(n)
        ]

    if reduce_outputs:
        # only a [8]-vector of per-shard sums crosses the axon relay: blocking
        # on the raw 16MB/core outputs marshals them to the client (~74ms/call)
        import jax.numpy as jnp
        inner = fn

        def _sums(*args):
            outs = inner(*args)
            return jax.jit(shard_map(
                lambda *os: tuple(jnp.sum(o, keepdims=True)[:, 0] for o in os),
                mesh=mesh,
                in_specs=(PartitionSpec("core"),) * len(outs),
                out_specs=(PartitionSpec("core"),) * len(outs),
                check_rep=False))(*outs)

        fn = _sums

    return fn, dev_in, make_zero_sets


def _time_pair(fnA, inA, mzA, fnB, inB, mzB, iters):
    """Interleaved A/B timing: returns median per-pair (tB - tA).

    The axon relay's ~75-125ms per-call overhead drifts on minute scales, so
    two sequential measurement blocks don't subtract cleanly — alternate the
    two programs and difference within each pair instead.
    """
    import time as _t
    import jax
    zA = mzA(iters + 1)
    zB = mzB(iters + 1)
    jax.block_until_ready(fnA(*inA, *zA[0]))
    jax.block_until_ready(fnB(*inB, *zB[0]))
    deltas, tAs = [], []
    for i in range(iters):
        t0 = _t.perf_counter()
        jax.block_until_ready(fnA(*inA, *zA[i + 1]))
        t1 = _t.perf_counter()
        jax.block_until_ready(fnB(*inB, *zB[i + 1]))
        t2 = _t.perf_counter()
        tAs.append(t1 - t0)
        deltas.append((t2 - t1) - (t1 - t0))
    deltas.sort()
    med = deltas[len(deltas) // 2]
    tAs.sort()
    return med, tAs[len(tAs) // 2]


def time_kernel(inputs, iters=20, passes=16):
    """Marginal per-pass device time: (t(passes) - t(1)) / (passes - 1).

    Cancels per-dispatch overhead (axon round trip, NEFF launch, transfers).
    """
    in_maps = _host_prep(**inputs)
    nc1 = _get_nc("silu", 1)
    fn1, in1, mz1 = _make_timed_fn(nc1, in_maps)
    ncK = _get_nc("silu", passes)
    fnK, inK, mzK = _make_timed_fn(ncK, in_maps)
    delta, t1 = _time_pair(fn1, in1, mz1, fnK, inK, mzK, iters)
    per_pass = delta / (passes - 1)
    print(f"t(1 pass)~{t1*1e6:.1f}us  median[t({passes})-t(1)]={delta*1e6:.1f}us  "
          f"marginal per-pass={per_pass*1e6:.1f}us")
    return per_pass * 1e9
